# revision 1
# baseline (speedup 1.0000x reference)
"""Trainium2 Bass kernel for nn_Bert_Proj_CRF (BERT projection + CRF NLL).

Strategy (data-parallel over batch, 8 NeuronCores x 8 sequences):
  - Embedding rows are gathered straight into x^T layout (d on partitions)
    with a bf16 transpose-gather (dma_gather), so the projection matmul
    out[s,t] = x @ (shared_W + domain_A[corpus_b]) needs no on-chip transpose.
  - log-softmax over T=4 on chip -> LS (log-probs) and q (probs).
  - CRF normalizer: the forward scan is an ordered product of 4x4 matrices
    M_t = exp(trans) * q_t (identity where masked).  We compute it as a
    chunked associative product: 64 chunks x 8 steps per sequence run in
    parallel across 512 lanes (linear space, rescaled), then a tree combine.
  - Gold path score via one-hot selections + ones-vector matmul reductions.
"""

import numpy as np
import ml_dtypes

import concourse.bass as bass
import concourse.bacc as bacc
import concourse.tile as tile
import concourse.mybir as mybir

V, D, T = 21128, 768, 4
B, S = 64, 512
NCORES = 8
BL = B // NCORES            # 8 sequences per core
NTOK = BL * S               # 4096 tokens per core
NG = NTOK // 128            # 32 token groups of 128
F32 = mybir.dt.float32
BF16 = mybir.dt.bfloat16
I16 = mybir.dt.int16
AF = mybir.ActivationFunctionType
AL = mybir.AluOpType
AX = mybir.AxisListType


def fap(t, off, dims):
    """AP over tile t's partition dim with custom free dims (element units)."""
    base = t if isinstance(t, bass.AP) else t[:]
    return bass.AP(
        tensor=base.tensor,
        offset=base.offset + off,
        ap=[list(base.ap[0])] + [list(d) for d in dims],
    )


def dap(handle, ap):
    return bass.AP(tensor=handle, offset=0, ap=[list(d) for d in ap])


_CACHE = {}
import os
KV = os.environ.get('KV', '')


def _build():
    if "nc" in _CACHE:
        return _CACHE["nc"]
    nc = bacc.Bacc()

    table_h = nc.dram_tensor("table", [V, D], BF16, kind="ExternalInput")
    gidx_h = nc.dram_tensor("gidx", [128, NG * 8], I16, kind="ExternalInput")
    hostf_h = nc.dram_tensor("hostf", [7, 128, NG], F32, kind="ExternalInput")
    hostp_h = nc.dram_tensor("hostp", [176], F32, kind="ExternalInput")
    a8t_h = nc.dram_tensor("a8t", [D, BL * T], F32, kind="ExternalInput")
    sw_h = nc.dram_tensor("sw", [D, T], F32, kind="ExternalInput")
    nll_h = nc.dram_tensor("nll", [BL], F32, kind="ExternalOutput")
    qd_h = nc.dram_tensor("qscratch", [NTOK * T], F32, kind="Internal")
    pzd_h = nc.dram_tensor("pzscratch", [128 * 21], F32, kind="Internal")
    gd_h = nc.dram_tensor("gscratch", [BL], F32, kind="Internal")

    with tile.TileContext(nc) as tc:
        with (
            tc.tile_pool(name="consts", bufs=1) as cp,
            tc.tile_pool(name="xt", bufs=3) as xp,
            tc.tile_pool(name="work", bufs=1) as wp,
            tc.tile_pool(name="psum", bufs=1, space="PSUM") as pp,
            tc.tile_pool(name="psum2", bufs=1, space="PSUM") as pp2,
        ):
            # ---------------- constants / inputs ----------------
            hostc = cp.tile([128, 7, NG], F32)
            nc.sync.dma_start(
                out=hostc[:],
                in_=dap(hostf_h, [[NG, 128], [128 * NG, 7], [1, NG]]),
            )
            words_mm = hostc[:, 0, :]
            target_mm = hostc[:, 1, :]
            tprev_mm = hostc[:, 2, :]
            sfirst_mm = hostc[:, 3, :]
            s0sel_mm = hostc[:, 4, :]
            words_sc = hostc[:, 5, :]
            sfirst_sc = hostc[:, 6, :]

            pc = cp.tile([128, 176], F32)
            nc.gpsimd.dma_start(out=pc[:], in_=dap(hostp_h, [[0, 128], [1, 176]]))
            db_bc = fap(pc, 0, [[1, 128]])            # [(b,s_hi,j)] = 128
            sb_bc = fap(pc, 128, [[1, 4]])
            trans_bc = fap(pc, 132, [[1, 16]])
            start_bc = fap(pc, 148, [[1, 4]])
            end_bc = fap(pc, 152, [[1, 4]])

            gidx = cp.tile([128, NG * 8], I16)
            nc.sync.dma_start(out=gidx[:], in_=gidx_h[:])

            a8t = cp.tile([128, 6, BL * T], F32)
            nc.sync.dma_start(
                out=a8t[:], in_=a8t_h[:].rearrange("(c p) j -> p c j", p=128)
            )
            swt = cp.tile([128, 6, T], F32)
            nc.sync.dma_start(
                out=swt[:], in_=sw_h[:].rearrange("(c p) j -> p c j", p=128)
            )
            w8f = cp.tile([128, 6, BL * T], F32)
            nc.vector.tensor_tensor(
                out=w8f[:],
                in0=a8t[:],
                in1=fap(swt, 0, [[4, 6], [0, BL], [1, T]]),
                op=AL.add,
            )
            w8b = cp.tile([128, 6, BL * T], BF16)
            nc.gpsimd.tensor_copy(out=w8b[:], in_=w8f[:])

            expT_bc = cp.tile([128, 16], F32)
            nc.scalar.activation(out=expT_bc[:], in_=trans_bc, func=AF.Exp)
            expS = cp.tile([128, T], F32)
            nc.scalar.activation(out=expS[:], in_=start_bc, func=AF.Exp)
            expE = cp.tile([128, T], F32)
            nc.scalar.activation(out=expE[:], in_=end_bc, func=AF.Exp)

            ones128 = cp.tile([128, 1], F32)
            nc.vector.memset(ones128[:], 1.0)
            ones1 = cp.tile([1, 128], F32)
            nc.vector.memset(ones1[:], 1.0)
            iota4 = cp.tile([128, T], F32)
            nc.gpsimd.iota(
                iota4[:], pattern=[[1, T]], base=0, channel_multiplier=0,
                allow_small_or_imprecise_dtypes=True,
            )
            iota16 = cp.tile([128, 16], F32)
            nc.gpsimd.iota(
                iota16[:], pattern=[[1, 16]], base=0, channel_multiplier=0,
                allow_small_or_imprecise_dtypes=True,
            )
            siota = cp.tile([128, NG], F32)
            nc.gpsimd.iota(
                siota[:], pattern=[[0, BL], [128, 4]], base=0, channel_multiplier=1,
                allow_small_or_imprecise_dtypes=True,
            )

            biasc = cp.tile([128, NG, T], F32)
            nc.vector.tensor_tensor(
                out=biasc[:], in0=fap(pc, 0, [[4, NG], [1, T]]),
                in1=fap(pc, 128, [[0, NG], [1, T]]), op=AL.add,
            )

            # ---------------- gather + projection matmuls ----------------
            lg_ps = pp.tile([128, NG, T], F32)
            if 'nofront' in KV:
                nc.vector.memset(lg_ps[:], 0.25)
            for b in range(BL if 'nofront' not in KV else 0):
                xt = xp.tile([128, 6, S], BF16, tag="xt")
                nc.gpsimd.dma_gather(
                    out_ap=xt[:],
                    in_ap=table_h[:],
                    idxs_ap=gidx[:, b * 32:(b + 1) * 32],
                    num_idxs=S,
                    num_idxs_reg=S,
                    elem_size=D,
                    transpose=True,
                )
                for gl in range(4):
                    for c in range(6):
                        nc.tensor.matmul(
                            lg_ps[:, b * 4 + gl, :],
                            lhsT=xt[:, c, gl * 128:(gl + 1) * 128],
                            rhs=w8b[:, c, b * T:(b + 1) * T],
                            start=(c == 0),
                            stop=(c == 5),
                        )

            # ---------------- softmax epilogue ----------------
            lp = wp.tile([128, NG, T], F32)
            nc.vector.tensor_tensor(out=lp[:], in0=lg_ps[:], in1=biasc[:], op=AL.add)
            mx = wp.tile([128, NG], F32)
            nc.vector.reduce_max(out=mx[:], in_=lp[:], axis=AX.X)
            sh = wp.tile([128, NG, T], F32)
            nc.vector.tensor_tensor(
                out=sh[:], in0=lp[:], in1=fap(mx, 0, [[1, NG], [0, T]]), op=AL.subtract
            )
            eu = wp.tile([128, NG, T], F32)
            nc.scalar.activation(out=eu[:], in_=sh[:], func=AF.Exp)
            sm = wp.tile([128, NG], F32)
            nc.vector.reduce_sum(out=sm[:], in_=eu[:], axis=AX.X)
            rs = wp.tile([128, NG], F32)
            nc.vector.reciprocal(out=rs[:], in_=sm[:])
            qq = wp.tile([128, NG, T], F32)
            nc.vector.tensor_tensor(
                out=qq[:], in0=eu[:], in1=fap(rs, 0, [[1, NG], [0, T]]), op=AL.mult
            )
            lns = wp.tile([128, NG], F32)
            nc.scalar.activation(out=lns[:], in_=sm[:], func=AF.Ln)
            LS = wp.tile([128, NG, T], F32)
            nc.vector.tensor_tensor(
                out=LS[:], in0=sh[:], in1=fap(lns, 0, [[1, NG], [0, T]]), op=AL.subtract
            )

            # q roundtrip through DRAM into scan layout
            nc.sync.dma_start(
                out=dap(qd_h, [[4, 128], [S * T, BL], [512, 4], [1, 4]]),
                in_=qq[:],
            )
            qscan = wp.tile([128, 128], F32)
            nc.sync.dma_start(
                out=qscan[:], in_=dap(qd_h, [[128, 128], [1, 128]])
            )

            # ---------------- gold path ----------------
            mask = wp.tile([128, NG], F32)
            nc.vector.tensor_scalar(
                out=mask[:], in0=words_mm, scalar1=0.0, scalar2=None,
                op0=AL.not_equal,
            )
            msk1 = wp.tile([128, NG], F32)
            nc.vector.tensor_tensor(out=msk1[:], in0=mask[:], in1=sfirst_mm, op=AL.mult)
            oh4 = wp.tile([128, NG, T], F32)
            nc.vector.tensor_tensor(
                out=oh4[:],
                in0=fap(hostc, NG, [[1, NG], [0, T]]),       # target_mm bcast over j
                in1=fap(iota4, 0, [[0, NG], [1, T]]),
                op=AL.is_equal,
            )
            em4 = wp.tile([128, NG, T], F32)
            nc.vector.tensor_tensor(out=em4[:], in0=LS[:], in1=oh4[:], op=AL.mult)
            emit = wp.tile([128, NG], F32)
            nc.vector.reduce_sum(out=emit[:], in_=em4[:], axis=AX.X)
            pair = wp.tile([128, NG], F32)
            nc.vector.tensor_scalar(
                out=pair[:], in0=tprev_mm, scalar1=4.0, scalar2=None, op0=AL.mult
            )
            nc.vector.tensor_tensor(out=pair[:], in0=pair[:], in1=target_mm, op=AL.add)
            oh16 = wp.tile([128, NG, 16], F32)
            nc.vector.tensor_tensor(
                out=oh16[:],
                in0=fap(pair, 0, [[1, NG], [0, 16]]),
                in1=fap(iota16, 0, [[0, NG], [1, 16]]),
                op=AL.is_equal,
            )
            trm = wp.tile([128, NG, 16], F32)
            nc.vector.tensor_tensor(
                out=trm[:], in0=oh16[:],
                in1=fap(pc, 132, [[0, NG], [1, 16]]), op=AL.mult,
            )
            tr = wp.tile([128, NG], F32)
            nc.vector.reduce_sum(out=tr[:], in_=trm[:], axis=AX.X)
            st4 = wp.tile([128, NG, T], F32)
            nc.vector.tensor_tensor(
                out=st4[:], in0=oh4[:], in1=fap(pc, 148, [[0, NG], [1, T]]), op=AL.mult
            )
            st = wp.tile([128, NG], F32)
            nc.vector.reduce_sum(out=st[:], in_=st4[:], axis=AX.X)
            nc.vector.tensor_tensor(out=st[:], in0=st[:], in1=s0sel_mm, op=AL.mult)
            e4 = wp.tile([128, NG, T], F32)
            nc.vector.tensor_tensor(
                out=e4[:], in0=oh4[:], in1=fap(pc, 152, [[0, NG], [1, T]]), op=AL.mult
            )
            eb = wp.tile([128, NG], F32)
            nc.vector.reduce_sum(out=eb[:], in_=e4[:], axis=AX.X)

            cnt_ps = pp2.tile([1, NG], F32)
            nc.tensor.matmul(cnt_ps[:], lhsT=ones128[:], rhs=mask[:], start=True, stop=True)
            cnt8 = wp.tile([1, BL], F32)
            nc.vector.reduce_sum(
                out=cnt8[:], in_=fap(cnt_ps, 0, [[4, BL], [1, 4]]), axis=AX.X
            )
            last8 = wp.tile([1, BL], F32)
            nc.vector.tensor_scalar(
                out=last8[:], in0=cnt8[:], scalar1=-1.0, scalar2=0.0,
                op0=AL.add, op1=AL.max,
            )
            lastrep = wp.tile([1, NG], F32)
            nc.vector.tensor_copy(out=lastrep[:], in_=fap(last8, 0, [[1, BL], [0, 4]]))
            lbc_ps = pp2.tile([128, NG], F32)
            nc.tensor.matmul(lbc_ps[:], lhsT=ones1[:], rhs=lastrep[:], start=True, stop=True)
            ind = wp.tile([128, NG], F32)
            nc.vector.tensor_tensor(out=ind[:], in0=siota[:], in1=lbc_ps[:], op=AL.is_equal)
            etok = wp.tile([128, NG], F32)
            nc.vector.tensor_tensor(out=etok[:], in0=eb[:], in1=ind[:], op=AL.mult)

            tt = wp.tile([128, NG], F32)
            nc.vector.tensor_tensor(out=tt[:], in0=emit[:], in1=mask[:], op=AL.mult)
            t2 = wp.tile([128, NG], F32)
            nc.vector.tensor_tensor(out=t2[:], in0=tr[:], in1=msk1[:], op=AL.mult)
            nc.vector.tensor_tensor(out=tt[:], in0=tt[:], in1=t2[:], op=AL.add)
            nc.vector.tensor_tensor(out=tt[:], in0=tt[:], in1=st[:], op=AL.add)
            nc.vector.tensor_tensor(out=tt[:], in0=tt[:], in1=etok[:], op=AL.add)
            gold_ps = pp2.tile([1, NG], F32)
            nc.tensor.matmul(gold_ps[:], lhsT=ones128[:], rhs=tt[:], start=True, stop=True)
            gold8 = wp.tile([1, BL], F32)
            nc.vector.reduce_sum(
                out=gold8[:], in_=fap(gold_ps, 0, [[4, BL], [1, 4]]), axis=AX.X
            )
            gold_p = wp.tile([BL, 1], F32)
            nc.sync.dma_start(out=gold_p[:], in_=gold8[:])

            # ---------------- CRF scan ----------------
            # Precompute all step matrices:
            #   Mf[cl, tau, k, j] = mb * expT[k, j] * q[cl, tau, j] + (1 - mb) * I
            mb = wp.tile([128, NG], F32)
            nc.vector.tensor_scalar(
                out=mb[:], in0=words_sc, scalar1=0.0, scalar2=None, op0=AL.not_equal
            )
            nc.vector.tensor_tensor(out=mb[:], in0=mb[:], in1=sfirst_sc, op=AL.mult)
            inv = wp.tile([128, NG], F32)
            nc.vector.tensor_scalar(
                out=inv[:], in0=mb[:], scalar1=-1.0, scalar2=1.0,
                op0=AL.mult, op1=AL.add,
            )
            Mf = wp.tile([128, NG, 16], F32)
            nc.vector.tensor_tensor(
                out=fap(Mf, 0, [[16, NG], [4, 4], [1, 4]]),
                in0=fap(qscan, 0, [[4, NG], [0, 4], [1, 4]]),
                in1=fap(expT_bc, 0, [[0, NG], [4, 4], [1, 4]]),
                op=AL.mult,
            )
            nc.vector.tensor_tensor(
                out=fap(Mf, 0, [[16, NG], [1, 16]]),
                in0=fap(Mf, 0, [[16, NG], [1, 16]]),
                in1=fap(mb, 0, [[1, NG], [0, 16]]),
                op=AL.mult,
            )
            nc.vector.tensor_tensor(
                out=fap(Mf, 0, [[16, NG], [5, 4]]),
                in0=fap(Mf, 0, [[16, NG], [5, 4]]),
                in1=fap(inv, 0, [[1, NG], [0, 4]]),
                op=AL.add,
            )

            # pairwise tree over the 8 step matrices of each chunk:
            # level A: 4 products per c_lo, level B: 2, level C: 1 -> Pst
            tmpA = wp.tile([128, 1024], F32)
            A2 = wp.tile([128, 16, 16], F32)     # (c_lo, pair) x 16
            nc.vector.tensor_tensor(
                out=fap(tmpA, 0, [[16, 64], [4, 4], [1, 4]]),
                in0=fap(Mf, 0, [[32, 16], [1, 16], [0, 4]]),
                in1=fap(Mf, 16, [[32, 16], [0, 4], [1, 16]]),
                op=AL.mult,
            )
            nc.vector.reduce_sum(
                out=fap(A2, 0, [[4, 64], [1, 4]]),
                in_=fap(tmpA, 0, [[16, 64], [1, 4], [4, 4]]),
                axis=AX.X,
            )
            B2 = wp.tile([128, 8, 16], F32)      # (c_lo, bp) x 16
            nc.vector.tensor_tensor(
                out=fap(tmpA, 0, [[16, 32], [4, 4], [1, 4]]),
                in0=fap(A2, 0, [[32, 8], [1, 16], [0, 4]]),
                in1=fap(A2, 16, [[32, 8], [0, 4], [1, 16]]),
                op=AL.mult,
            )
            nc.vector.reduce_sum(
                out=fap(B2, 0, [[4, 32], [1, 4]]),
                in_=fap(tmpA, 0, [[16, 32], [1, 4], [4, 4]]),
                axis=AX.X,
            )
            Pst = wp.tile([128, 4, 16], F32)
            nc.vector.tensor_tensor(
                out=fap(tmpA, 0, [[16, 16], [4, 4], [1, 4]]),
                in0=fap(B2, 0, [[32, 4], [1, 16], [0, 4]]),
                in1=fap(B2, 16, [[32, 4], [0, 4], [1, 16]]),
                op=AL.mult,
            )
            nc.vector.reduce_sum(
                out=fap(Pst, 0, [[4, 16], [1, 4]]),
                in_=fap(tmpA, 0, [[16, 16], [1, 4], [4, 4]]),
                axis=AX.X,
            )
            # combine level 1: pairs over c_lo -> P2 [128, 2, 16]
            # iter (pair, i, k, j); tmp2 layout [pair][i][k][j]
            tmp2 = wp.tile([128, 128], F32)
            nc.vector.tensor_tensor(
                out=fap(tmp2, 0, [[16, 8], [4, 4], [1, 4]]),
                in0=fap(Pst, 0, [[32, 2], [1, 16], [0, 4]]),
                in1=fap(Pst, 16, [[32, 2], [0, 4], [1, 16]]),
                op=AL.mult,
            )
            P2 = wp.tile([128, 2, 16], F32)
            nc.vector.reduce_sum(
                out=fap(P2, 0, [[4, 8], [1, 4]]),
                in_=fap(tmp2, 0, [[16, 8], [1, 4], [4, 4]]),
                axis=AX.X,
            )
            # combine level 2 -> P4z[:, 0:16], lacc -> P4z[:, 16], q0 -> P4z[:, 17:21]
            P4z = wp.tile([128, 21], F32)
            tmp3 = wp.tile([128, 64], F32)
            # iter (i, k, j); tmp3 layout [i][k][j]
            nc.vector.tensor_tensor(
                out=fap(tmp3, 0, [[16, 4], [4, 4], [1, 4]]),
                in0=fap(P2, 0, [[1, 16], [0, 4]]),
                in1=fap(P2, 16, [[0, 4], [1, 16]]),
                op=AL.mult,
            )
            nc.vector.reduce_sum(
                out=fap(P4z, 0, [[4, 4], [1, 4]]),
                in_=fap(tmp3, 0, [[16, 4], [1, 4], [4, 4]]),
                axis=AX.X,
            )
            m1 = wp.tile([128, 1], F32)
            nc.vector.reduce_max(out=m1[:], in_=fap(P4z, 0, [[1, 16]]), axis=AX.X)
            rm1 = wp.tile([128, 1], F32)
            nc.vector.reciprocal(out=rm1[:], in_=m1[:])
            nc.vector.tensor_tensor(
                out=fap(P4z, 0, [[1, 16]]), in0=fap(P4z, 0, [[1, 16]]),
                in1=fap(rm1, 0, [[0, 16]]), op=AL.mult,
            )
            nc.scalar.activation(
                out=fap(P4z, 16, [[1, 1]]), in_=m1[:], func=AF.Ln
            )
            nc.vector.tensor_copy(out=fap(P4z, 17, [[1, 4]]), in_=qscan[:, 0:4])

            # fold partitions via DRAM roundtrip: [128, 21] -> [8, 16*21]
            Pz = wp.tile([BL, 16 * 21], F32)
            nc.sync.dma_start(out=Pz[:], in_=P4z[:])

            # combine levels 3-6 (free dim), no per-level rescale
            # each: iter (pair, i, k, j); t3 layout [pair][i][k][j]
            t3 = wp.tile([BL, 512], F32)
            P3 = wp.tile([BL, 8, 16], F32)
            nc.vector.tensor_tensor(
                out=fap(t3, 0, [[16, 32], [4, 4], [1, 4]]),
                in0=fap(Pz, 0, [[42, 8], [1, 16], [0, 4]]),
                in1=fap(Pz, 21, [[42, 8], [0, 4], [1, 16]]),
                op=AL.mult,
            )
            nc.vector.reduce_sum(
                out=fap(P3, 0, [[4, 32], [1, 4]]),
                in_=fap(t3, 0, [[16, 32], [1, 4], [4, 4]]),
                axis=AX.X,
            )
            P4b = wp.tile([BL, 4, 16], F32)
            nc.vector.tensor_tensor(
                out=fap(t3, 0, [[16, 16], [4, 4], [1, 4]]),
                in0=fap(P3, 0, [[32, 4], [1, 16], [0, 4]]),
                in1=fap(P3, 16, [[32, 4], [0, 4], [1, 16]]),
                op=AL.mult,
            )
            nc.vector.reduce_sum(
                out=fap(P4b, 0, [[4, 16], [1, 4]]),
                in_=fap(t3, 0, [[16, 16], [1, 4], [4, 4]]),
                axis=AX.X,
            )
            P5 = wp.tile([BL, 2, 16], F32)
            nc.vector.tensor_tensor(
                out=fap(t3, 0, [[16, 8], [4, 4], [1, 4]]),
                in0=fap(P4b, 0, [[32, 2], [1, 16], [0, 4]]),
                in1=fap(P4b, 16, [[32, 2], [0, 4], [1, 16]]),
                op=AL.mult,
            )
            nc.vector.reduce_sum(
                out=fap(P5, 0, [[4, 8], [1, 4]]),
                in_=fap(t3, 0, [[16, 8], [1, 4], [4, 4]]),
                axis=AX.X,
            )
            Ptot = wp.tile([BL, 16], F32)
            nc.vector.tensor_tensor(
                out=fap(t3, 0, [[16, 4], [4, 4], [1, 4]]),
                in0=fap(P5, 0, [[1, 16], [0, 4]]),
                in1=fap(P5, 16, [[0, 4], [1, 16]]),
                op=AL.mult,
            )
            nc.vector.reduce_sum(
                out=fap(Ptot, 0, [[4, 4], [1, 4]]),
                in_=fap(t3, 0, [[16, 4], [1, 4], [4, 4]]),
                axis=AX.X,
            )
            laccs = wp.tile([BL, 1], F32)
            nc.vector.reduce_sum(out=laccs[:], in_=fap(Pz, 16, [[21, 16]]), axis=AX.X)

            # final: alpha = (q0 * expS) @ Ptot ; Z = sum(alpha * expE)
            q0s = wp.tile([BL, T], F32)
            nc.vector.tensor_tensor(
                out=q0s[:], in0=fap(Pz, 17, [[1, 4]]), in1=expS[0:BL, :], op=AL.mult
            )
            ta = wp.tile([BL, 16], F32)
            nc.vector.tensor_tensor(
                out=fap(ta, 0, [[4, 4], [1, 4]]),
                in0=fap(q0s, 0, [[0, 4], [1, 4]]),
                in1=fap(Ptot, 0, [[1, 4], [4, 4]]),
                op=AL.mult,
            )
            av = wp.tile([BL, T], F32)
            nc.vector.reduce_sum(out=av[:], in_=fap(ta, 0, [[4, 4], [1, 4]]), axis=AX.X)
            ze = wp.tile([BL, T], F32)
            nc.vector.tensor_tensor(out=ze[:], in0=av[:], in1=expE[0:BL, :], op=AL.mult)
            Z = wp.tile([BL, 1], F32)
            nc.vector.reduce_sum(out=Z[:], in_=ze[:], axis=AX.X)
            lnZ = wp.tile([BL, 1], F32)
            nc.scalar.activation(out=lnZ[:], in_=Z[:], func=AF.Ln)
            norm = wp.tile([BL, 1], F32)
            nc.vector.tensor_tensor(out=norm[:], in0=lnZ[:], in1=laccs[:], op=AL.add)
            nllp = wp.tile([BL, 1], F32)
            nc.vector.tensor_tensor(out=nllp[:], in0=norm[:], in1=gold_p[:], op=AL.subtract)
            nc.sync.dma_start(out=nll_h[:], in_=nllp[:])

    nc.compile()
    _CACHE["nc"] = nc
    return nc


def _prep_core(words, target, corpus, shared_b, domain_A, domain_b,
               trans_m, start_scores, end_scores):
    w = np.asarray(words, np.int64)
    t = np.asarray(target, np.int64)

    def mm(a):
        return np.ascontiguousarray(
            np.asarray(a, np.float64).reshape(BL, 4, 128)
            .transpose(2, 0, 1).reshape(128, NG)
        ).astype(np.float32)

    def sc(a):
        return np.ascontiguousarray(
            np.asarray(a, np.float64).reshape(BL, 16, 4, 8).reshape(128, 32)
        ).astype(np.float32)

    sfirst = np.ones((BL, S)); sfirst[:, 0] = 0.0
    tprev = np.concatenate([np.zeros((BL, 1), np.int64), t[:, :-1]], axis=1)
    hostf = np.stack([
        mm(w), mm(t), mm(tprev), mm(sfirst), mm(1.0 - sfirst), sc(w), sc(sfirst),
    ]).astype(np.float32)

    gidx = np.zeros((128, NG * 8), np.int16)
    for b in range(BL):
        gidx[:16, b * 32:(b + 1) * 32] = w[b].reshape(32, 16).T.astype(np.int16)

    hostp = np.zeros(176, np.float32)
    hostp[0:128] = np.repeat(
        np.asarray(domain_b, np.float32)[corpus][:, None, :], 4, axis=1
    ).reshape(-1)
    hostp[128:132] = np.asarray(shared_b, np.float32)
    hostp[132:148] = np.asarray(trans_m, np.float32).reshape(-1)
    hostp[148:152] = np.asarray(start_scores, np.float32)
    hostp[152:156] = np.asarray(end_scores, np.float32)

    a8t = np.ascontiguousarray(
        np.asarray(domain_A, np.float32)[corpus].transpose(1, 0, 2).reshape(D, BL * T)
    )
    return hostf, gidx, hostp, a8t


def kernel(_trace=False, **inputs):
    from concourse.bass_utils import run_bass_kernel_spmd

    words = np.asarray(inputs["words"])
    target = np.asarray(inputs["target"])
    corpus = np.asarray(inputs["corpus"])
    table_bf16 = np.ascontiguousarray(
        np.asarray(inputs["embed_table"], np.float32).astype(ml_dtypes.bfloat16)
    )
    sw = np.ascontiguousarray(np.asarray(inputs["shared_W"], np.float32))

    nc = _build()
    in_maps = []
    for k in range(NCORES):
        sl = slice(k * BL, (k + 1) * BL)
        hostf, gidx, hostp, a8t = _prep_core(
            words[sl], target[sl], corpus[sl], inputs["shared_b"],
            inputs["domain_A"], inputs["domain_b"], inputs["trans_m"],
            inputs["start_scores"], inputs["end_scores"],
        )
        in_maps.append({
            "table": table_bf16, "gidx": gidx, "hostf": hostf,
            "hostp": hostp, "a8t": a8t, "sw": sw,
        })
    res = run_bass_kernel_spmd(
        nc, in_maps, core_ids=list(range(NCORES)), trace=_trace,
    )
    if _trace:
        print("exec_time_ns:", res.exec_time_ns,
              "mean:", res.mean_exec_time_ns,
              "trace:", (res.instructions_and_trace or (None, None))[1])
    out = np.concatenate([res.results[k]["nll"] for k in range(NCORES)])
    return out.astype(np.float32)



# revision 3
# speedup vs baseline: 1.4081x; 1.4081x over previous
"""Trainium2 Bass kernel for nn_Bert_Proj_CRF (BERT projection + CRF NLL).

v2 design (data-parallel, 8 cores x 8 sequences):
  - fp8 embedding table; transpose-gather with a PERMUTED token order so
    PSUM partition p holds 4 consecutive steps (s = 4p+gl) of each
    sequence -> the CRF chunk layout needs no DRAM shuffle at all.
  - Bias folded into PSUM via a ones-outer-product matmul; the scan runs
    on UNNORMALIZED exp(logits) (the per-token log-sum-exp terms cancel
    against the gold emission score), so no per-token softmax division
    and no per-token Ln.
  - Per-sequence pipeline: gather_b || {matmuls, exp, chunk-matrix build,
    2 in-lane tree levels, gold partials} of earlier sequences.
  - Cross-chunk combine: per-b SBUF fold (128 lanes -> 8 lanes of a
    64-lane tile), 4 tree levels, second fold to 8 lanes, 3 levels,
    then the final alpha/Z assembly. One rescale (after 32-step
    products) keeps f32 in range; its logs ride the folds.
"""

import os
import numpy as np
import ml_dtypes

import concourse.bass as bass
import concourse.bacc as bacc
import concourse.tile as tile
import concourse.mybir as mybir

V, D, T = 21128, 768, 4
B, S = 64, 512
NCORES = 8
BL = B // NCORES            # 8 sequences per core
NG = 32                     # (b, gl) pairs: bg = b*4 + gl
F32 = mybir.dt.float32
FP8 = mybir.dt.float8e4
I16 = mybir.dt.int16
AF = mybir.ActivationFunctionType
AL = mybir.AluOpType
AX = mybir.AxisListType

KV = os.environ.get('KV', '')
PLAIN8 = 'plain8' in KV     # flip if the fp8 transpose-gather is NOT 16-bit interleaved

# hostK column layout (f32, [128, HK])
HK_TRANS = 0     # 16: trans matrix (broadcast all partitions)
HK_START = 16    # 4
HK_END = 20      # 4
HK_C0M = 24      # 1: per-lane b (lanes 0..7): 1 - mask0_b
HK_MB = 25       # 32: mask*(s>=1) per (p, bg)
HK_OH4M = 57     # 128: onehot(target)*mask per (p, bg, t)
HK_GPRE = 185    # 32: host-gathered trans/start/end gold terms per (p, bg)
HK_INVI = 217    # 512: (1-mb)*(k==j) per (p, bg, k, j)
HK_BIAS = 729    # 128: row 0 only: bias per (b, gl, t) (same for all gl)
HK_ONESROW = 857 # 128: row 0 only: 1.0 (bias-matmul lhsT)
HK_ONESCOL = 985 # 1: all partitions: 1.0 (gold-matmul lhsT)
HK = 992


def fap(t, off, dims):
    """AP over tile t's partition dim with custom free dims (element units)."""
    base = t if isinstance(t, bass.AP) else t[:]
    return bass.AP(
        tensor=base.tensor,
        offset=base.offset + off,
        ap=[list(base.ap[0])] + [list(d) for d in dims],
    )


_CACHE = {}


def _build():
    if "nc" in _CACHE:
        return _CACHE["nc"]
    nc = bacc.Bacc()

    table_h = nc.dram_tensor("table", [V, D], FP8, kind="ExternalInput")
    gidx_h = nc.dram_tensor("gidx", [128, BL * 32], I16, kind="ExternalInput")
    hostk_h = nc.dram_tensor("hostk", [128, HK], F32, kind="ExternalInput")
    rhs8_h = nc.dram_tensor("rhs8", [128, 6 * NG], FP8, kind="ExternalInput")
    nll_h = nc.dram_tensor("nll", [BL], F32, kind="ExternalOutput")

    with tile.TileContext(nc) as tc:
        with (
            tc.tile_pool(name="consts", bufs=1) as cp,
            tc.tile_pool(name="xt", bufs=3) as xp,
            tc.tile_pool(name="work", bufs=1) as wp,
            tc.tile_pool(name="psum", bufs=1, space="PSUM") as pp,
            tc.tile_pool(name="psum2", bufs=1, space="PSUM") as pp2,
        ):
            # ---------------- t0 const loads (SP, in need-order) -------------
            gidx = cp.tile([128, BL * 32], I16)
            nc.sync.dma_start(out=gidx[:], in_=gidx_h[:])
            hostk = cp.tile([128, HK], F32)
            nc.sync.dma_start(out=hostk[:], in_=hostk_h[:])
            rhs8 = cp.tile([128, 6 * NG], FP8)
            nc.sync.dma_start(out=rhs8[:], in_=rhs8_h[:])

            # Act: exp of the small params (Exp table loads here, off-path)
            expT = wp.tile([128, 16], F32)
            nc.scalar.activation(out=expT[:], in_=fap(hostk, HK_TRANS, [[1, 16]]),
                                 func=AF.Exp)
            expS8 = wp.tile([BL, T], F32)
            nc.scalar.activation(out=expS8[:], in_=fap(hostk, HK_START, [[1, 4]])[0:BL],
                                 func=AF.Exp)
            expE8 = wp.tile([BL, T], F32)
            nc.scalar.activation(out=expE8[:], in_=fap(hostk, HK_END, [[1, 4]])[0:BL],
                                 func=AF.Exp)

            # DVE: mbexpT[p, bg, k, j] = mb[p,bg] * expT[k,j]
            mbexpT = wp.tile([128, NG * 16], F32)
            nc.vector.tensor_tensor(
                out=fap(mbexpT, 0, [[16, NG], [1, 16]]),
                in0=fap(hostk, HK_MB, [[1, NG], [0, 16]]),
                in1=fap(expT, 0, [[0, NG], [1, 16]]),
                op=AL.mult,
            )

            # PE: bias broadcast into PSUM: lg[p, bg*4+t] = bias[bg*4+t]
            lg_ps = pp.tile([128, NG * T], F32)
            nc.tensor.matmul(
                lg_ps[:],
                lhsT=fap(hostk, HK_ONESROW, [[1, 128]])[0:1],
                rhs=fap(hostk, HK_BIAS, [[1, 128]])[0:1],
                start=True, stop=False, skip_group_check=True,
            )

            # ---------------- per-sequence pipeline ----------------
            eu = wp.tile([128, NG * T], F32)       # exp(logits), scan emissions
            P1 = wp.tile([128, BL * 16], F32)      # 4-step chunk products
            F2 = wp.tile([64, 16 * 16], F32)       # fold1 dest: lane (b,g), (j, ij)
            Mf = wp.tile([128, 64], F32)
            tmpA = wp.tile([128, 128], F32)
            A2 = wp.tile([128, 32], F32)
            tmp2 = wp.tile([128, 64], F32)
            em4 = wp.tile([128, 16], F32)
            emitk = wp.tile([128, 4], F32)
            ttb = wp.tile([128, 4], F32)
            gold_ps = pp2.tile([1, NG], F32)

            for b in range(BL):
                xt = xp.tile([128, 6, S], FP8, tag="xt")
                nc.gpsimd.dma_gather(
                    out_ap=xt[:],
                    in_ap=table_h[:],
                    idxs_ap=gidx[:, b * 32:(b + 1) * 32],
                    num_idxs=S,
                    num_idxs_reg=S,
                    elem_size=D,
                    transpose=True,
                )
                for gl in range(4):
                    og = fap(lg_ps, (b * 4 + gl) * 4, [[1, 4]])
                    if PLAIN8:
                        for c in range(6):
                            nc.tensor.matmul(
                                og,
                                lhsT=fap(xt, c * 512 + gl * 128, [[1, 128]]),
                                rhs=fap(rhs8, c * NG + b * 4, [[1, 4]]),
                                start=False, stop=(c == 5), skip_group_check=True,
                            )
                    else:
                        for c2 in range(3):
                            for e in range(2):
                                nc.tensor.matmul(
                                    og,
                                    lhsT=fap(xt, c2 * 1024 + gl * 256 + e, [[2, 128]]),
                                    rhs=fap(rhs8, (2 * c2 + e) * NG + b * 4, [[1, 4]]),
                                    start=False, stop=(c2 == 2 and e == 1),
                                    skip_group_check=True,
                                )
                # Act: eu_b = exp(lp_b)  (unnormalized; includes bias)
                nc.scalar.activation(
                    out=fap(eu, b * 16, [[1, 16]]),
                    in_=fap(lg_ps, b * 16, [[1, 16]]),
                    func=AF.Exp,
                )
                # DVE: step matrices M[gl][k,j] = mbexpT * eu[gl, j] + invI
                nc.vector.tensor_tensor(
                    out=fap(Mf, 0, [[16, 4], [4, 4], [1, 4]]),
                    in0=fap(mbexpT, b * 64, [[16, 4], [4, 4], [1, 4]]),
                    in1=fap(eu, b * 16, [[4, 4], [0, 4], [1, 4]]),
                    op=AL.mult,
                )
                nc.vector.tensor_tensor(
                    out=fap(Mf, 0, [[16, 4], [1, 16]]),
                    in0=fap(Mf, 0, [[16, 4], [1, 16]]),
                    in1=fap(hostk, HK_INVI + b * 64, [[16, 4], [1, 16]]),
                    op=AL.add,
                )
                # L1: two pair products (M0*M1, M2*M3)
                nc.vector.tensor_tensor(
                    out=fap(tmpA, 0, [[16, 8], [4, 4], [1, 4]]),
                    in0=fap(Mf, 0, [[32, 2], [1, 16], [0, 4]]),
                    in1=fap(Mf, 16, [[32, 2], [0, 4], [1, 16]]),
                    op=AL.mult,
                )
                nc.vector.reduce_sum(
                    out=fap(A2, 0, [[4, 8], [1, 4]]),
                    in_=fap(tmpA, 0, [[16, 8], [1, 4], [4, 4]]),
                    axis=AX.X,
                )
                # L2: chunk product -> P1[:, b*16:(b+1)*16]
                nc.vector.tensor_tensor(
                    out=fap(tmp2, 0, [[16, 4], [4, 4], [1, 4]]),
                    in0=fap(A2, 0, [[32, 1], [1, 16], [0, 4]]),
                    in1=fap(A2, 16, [[32, 1], [0, 4], [1, 16]]),
                    op=AL.mult,
                )
                nc.vector.reduce_sum(
                    out=fap(P1, b * 16, [[4, 4], [1, 4]]),
                    in_=fap(tmp2, 0, [[16, 4], [1, 4], [4, 4]]),
                    axis=AX.X,
                )
                # gold partials: emit = sum_t lp*oh4m, + host-gathered terms
                nc.vector.tensor_tensor(
                    out=em4[:],
                    in0=fap(lg_ps, b * 16, [[1, 16]]),
                    in1=fap(hostk, HK_OH4M + b * 16, [[1, 16]]),
                    op=AL.mult,
                )
                nc.vector.reduce_sum(
                    out=emitk[:], in_=fap(em4, 0, [[4, 4], [1, 4]]), axis=AX.X,
                )
                nc.vector.tensor_tensor(
                    out=ttb[:], in0=emitk[:],
                    in1=fap(hostk, HK_GPRE + b * 4, [[1, 4]]),
                    op=AL.add,
                )
                # SP: fold chunk products of seq b into F2 lanes b*8..b*8+8
                nc.sync.dma_start(
                    out=fap(F2, 0, [[16, 16], [1, 16]])[b * 8:(b + 1) * 8],
                    in_=fap(P1, b * 16, [[1, 16]]),
                )
                # PE: gold accumulate (after data mms; PE is in-order)
                nc.tensor.matmul(
                    fap(gold_ps, b * 4, [[1, 4]]),
                    lhsT=fap(hostk, HK_ONESCOL, [[1, 1]]),
                    rhs=ttb[:],
                    start=True, stop=True, skip_group_check=True,
                )

            # ---------------- endgame ----------------
            # Act: prefetch the Ln table while folds/trees run
            lnscr = wp.tile([1, 1], F32)
            nc.scalar.activation(
                out=lnscr[:], in_=fap(hostk, HK_ONESCOL, [[1, 1]])[0:1], func=AF.Ln)

            # Pool: eu0 rows + gold row folds (Pool is free after gathers)
            eu0T = wp.tile([BL, 4], F32)
            nc.gpsimd.dma_start(
                out=eu0T[:], in_=fap(eu, 0, [[16, 8], [1, 4]])[0:1])
            gold8 = wp.tile([1, BL], F32)
            nc.vector.reduce_sum(
                out=gold8[:], in_=fap(gold_ps, 0, [[4, 8], [1, 4]]), axis=AX.X)
            goldT = wp.tile([BL, 1], F32)
            nc.gpsimd.dma_start(out=goldT[:], in_=gold8[:])

            # phase2 on F2 [64 lanes = (b,g), 16 matrices each]
            t2 = wp.tile([64, 512], F32)
            G8 = wp.tile([64, 128], F32)
            nc.vector.tensor_tensor(
                out=fap(t2, 0, [[16, 32], [4, 4], [1, 4]]),
                in0=fap(F2, 0, [[32, 8], [1, 16], [0, 4]]),
                in1=fap(F2, 16, [[32, 8], [0, 4], [1, 16]]),
                op=AL.mult,
            )
            nc.vector.reduce_sum(
                out=fap(G8, 0, [[4, 32], [1, 4]]),
                in_=fap(t2, 0, [[16, 32], [1, 4], [4, 4]]),
                axis=AX.X,
            )
            G4 = wp.tile([64, 64], F32)
            nc.vector.tensor_tensor(
                out=fap(t2, 0, [[16, 16], [4, 4], [1, 4]]),
                in0=fap(G8, 0, [[32, 4], [1, 16], [0, 4]]),
                in1=fap(G8, 16, [[32, 4], [0, 4], [1, 16]]),
                op=AL.mult,
            )
            nc.vector.reduce_sum(
                out=fap(G4, 0, [[4, 16], [1, 4]]),
                in_=fap(t2, 0, [[16, 16], [1, 4], [4, 4]]),
                axis=AX.X,
            )
            G2 = wp.tile([64, 32], F32)
            nc.vector.tensor_tensor(
                out=fap(t2, 0, [[16, 8], [4, 4], [1, 4]]),
                in0=fap(G4, 0, [[32, 2], [1, 16], [0, 4]]),
                in1=fap(G4, 16, [[32, 2], [0, 4], [1, 16]]),
                op=AL.mult,
            )
            nc.vector.reduce_sum(
                out=fap(G2, 0, [[4, 8], [1, 4]]),
                in_=fap(t2, 0, [[16, 8], [1, 4], [4, 4]]),
                axis=AX.X,
            )
            # rescale the two 32-step products per lane; log the maxes
            rmax = wp.tile([64, 2], F32)
            nc.vector.reduce_max(
                out=rmax[:], in_=fap(G2, 0, [[16, 2], [1, 16]]), axis=AX.X)
            rrec = wp.tile([64, 2], F32)
            nc.vector.reciprocal(out=rrec[:], in_=rmax[:])
            nc.vector.tensor_tensor(
                out=fap(G2, 0, [[16, 2], [1, 16]]),
                in0=fap(G2, 0, [[16, 2], [1, 16]]),
                in1=fap(rrec, 0, [[1, 2], [0, 16]]),
                op=AL.mult,
            )
            lgs = wp.tile([64, 2], F32)
            nc.scalar.activation(out=lgs[:], in_=rmax[:], func=AF.Ln)
            # L4 -> G17[:, 0:16]; log sum -> G17[:, 16]
            G17 = wp.tile([64, 17], F32)
            nc.vector.tensor_tensor(
                out=fap(t2, 0, [[16, 4], [4, 4], [1, 4]]),
                in0=fap(G2, 0, [[1, 16], [0, 4]]),
                in1=fap(G2, 16, [[0, 4], [1, 16]]),
                op=AL.mult,
            )
            nc.vector.reduce_sum(
                out=fap(G17, 0, [[4, 4], [1, 4]]),
                in_=fap(t2, 0, [[16, 4], [1, 4], [4, 4]]),
                axis=AX.X,
            )
            nc.vector.reduce_sum(
                out=fap(G17, 16, [[1, 1]]), in_=lgs[:], axis=AX.X)

            # SP: fold2 -> F3 [8 lanes, 8 groups x 17]
            F3 = wp.tile([BL, 8 * 17], F32)
            nc.sync.dma_start(out=F3[:], in_=G17[:])

            # phase3 on F3
            t3 = wp.tile([BL, 256], F32)
            H4 = wp.tile([BL, 64], F32)
            nc.vector.tensor_tensor(
                out=fap(t3, 0, [[16, 16], [4, 4], [1, 4]]),
                in0=fap(F3, 0, [[34, 4], [1, 16], [0, 4]]),
                in1=fap(F3, 17, [[34, 4], [0, 4], [1, 16]]),
                op=AL.mult,
            )
            nc.vector.reduce_sum(
                out=fap(H4, 0, [[4, 16], [1, 4]]),
                in_=fap(t3, 0, [[16, 16], [1, 4], [4, 4]]),
                axis=AX.X,
            )
            H2 = wp.tile([BL, 32], F32)
            nc.vector.tensor_tensor(
                out=fap(t3, 0, [[16, 8], [4, 4], [1, 4]]),
                in0=fap(H4, 0, [[32, 2], [1, 16], [0, 4]]),
                in1=fap(H4, 16, [[32, 2], [0, 4], [1, 16]]),
                op=AL.mult,
            )
            nc.vector.reduce_sum(
                out=fap(H2, 0, [[4, 8], [1, 4]]),
                in_=fap(t3, 0, [[16, 8], [1, 4], [4, 4]]),
                axis=AX.X,
            )
            Ht = wp.tile([BL, 16], F32)
            nc.vector.tensor_tensor(
                out=fap(t3, 0, [[16, 4], [4, 4], [1, 4]]),
                in0=fap(H2, 0, [[1, 16], [0, 4]]),
                in1=fap(H2, 16, [[0, 4], [1, 16]]),
                op=AL.mult,
            )
            nc.vector.reduce_sum(
                out=fap(Ht, 0, [[4, 4], [1, 4]]),
                in_=fap(t3, 0, [[16, 4], [1, 4], [4, 4]]),
                axis=AX.X,
            )
            lgsum3 = wp.tile([BL, 1], F32)
            nc.vector.reduce_sum(
                out=lgsum3[:], in_=fap(F3, 16, [[17, 8]]), axis=AX.X)

            # final: alpha0 = eu0*expS; Z = (alpha0 @ Htot) . expE
            a0 = wp.tile([BL, 4], F32)
            nc.vector.tensor_tensor(out=a0[:], in0=eu0T[:], in1=expS8[:], op=AL.mult)
            ta = wp.tile([BL, 16], F32)
            nc.vector.tensor_tensor(
                out=fap(ta, 0, [[4, 4], [1, 4]]),        # [j, k]
                in0=fap(a0, 0, [[0, 4], [1, 4]]),
                in1=fap(Ht, 0, [[1, 4], [4, 4]]),
                op=AL.mult,
            )
            av = wp.tile([BL, 4], F32)
            nc.vector.reduce_sum(
                out=av[:], in_=fap(ta, 0, [[4, 4], [1, 4]]), axis=AX.X)
            ze = wp.tile([BL, 4], F32)
            nc.vector.tensor_tensor(out=ze[:], in0=av[:], in1=expE8[:], op=AL.mult)
            Zt = wp.tile([BL, 1], F32)
            nc.vector.reduce_sum(out=Zt[:], in_=ze[:], axis=AX.X)
            # mask0 correction: (1-mask0) * ln(sum eu0)
            sm0 = wp.tile([BL, 1], F32)
            nc.vector.reduce_sum(out=sm0[:], in_=eu0T[:], axis=AX.X)
            lnsm0 = wp.tile([BL, 1], F32)
            nc.scalar.activation(out=lnsm0[:], in_=sm0[:], func=AF.Ln)
            c0c = wp.tile([BL, 1], F32)
            nc.vector.tensor_tensor(
                out=c0c[:], in0=lnsm0[:],
                in1=fap(hostk, HK_C0M, [[1, 1]])[0:BL], op=AL.mult)
            lnZ = wp.tile([BL, 1], F32)
            nc.scalar.activation(out=lnZ[:], in_=Zt[:], func=AF.Ln)
            norm = wp.tile([BL, 1], F32)
            nc.vector.tensor_tensor(out=norm[:], in0=lnZ[:], in1=lgsum3[:], op=AL.add)
            nllp = wp.tile([BL, 1], F32)
            nc.vector.tensor_tensor(out=nllp[:], in0=norm[:], in1=goldT[:],
                                    op=AL.subtract)
            nc.vector.tensor_tensor(out=nllp[:], in0=nllp[:], in1=c0c[:],
                                    op=AL.subtract)
            nc.sync.dma_start(out=nll_h[:], in_=nllp[:])

    nc.compile()
    _CACHE["nc"] = nc
    return nc


def _prep_core(words, target, corpus, embed_f32, shared_W, shared_b,
               domain_A, domain_b, trans_m, start_scores, end_scores):
    w = np.asarray(words, np.int64)          # [BL, S]
    t = np.asarray(target, np.int64)

    # permuted gather order: position k <-> token s = 4*(k%128) + k//128
    kk = np.arange(S)
    perm = 4 * (kk % 128) + kk // 128        # s for each position k
    gidx = np.zeros((128, BL * 32), np.int16)
    for b in range(BL):
        il = w[b, perm].astype(np.int16)     # idxs[k]
        gidx[:16, b * 32:(b + 1) * 32] = il.reshape(32, 16).T

    # per-(p, bg) token tensors, s = 4p + gl
    # layout [p, b, gl]: a[b, 4p+gl] -> reshape(BL, 128, 4) transpose(1,0,2)
    def pm(a):
        return np.ascontiguousarray(
            np.asarray(a, np.float64).reshape(BL, 128, 4)
            .transpose(1, 0, 2).reshape(128, NG)).astype(np.float32)

    mask = (w != 0)
    sfirst = np.ones((BL, S)); sfirst[:, 0] = 0.0
    mb = pm(mask * sfirst)                   # [128, NG]

    # gold host-gathered terms
    trans = np.asarray(trans_m, np.float64)
    start = np.asarray(start_scores, np.float64)
    end = np.asarray(end_scores, np.float64)
    tr_vals = trans[t[:, :-1], t[:, 1:]] * mask[:, 1:]          # [BL, S-1]
    gpre_tok = np.zeros((BL, S))
    gpre_tok[:, 1:] += tr_vals
    gpre_tok[:, 0] += start[t[:, 0]]
    last_idx = np.maximum(mask.sum(1) - 1, 0)
    bidx = np.arange(BL)
    gpre_tok[bidx, last_idx] += end[t[bidx, last_idx]]
    gpre = pm(gpre_tok)

    # one-hot(target)*mask [p, bg, t]
    oh = (t[..., None] == np.arange(T)[None, None, :]) * mask[..., None]
    oh4m = np.ascontiguousarray(
        oh.reshape(BL, 128, 4, T).transpose(1, 0, 2, 3)
        .reshape(128, NG * T)).astype(np.float32)

    # invI [p, bg, k, j] = (1-mb)*(k==j)
    eye = np.eye(T).reshape(1, 1, T * T)
    invI = ((1.0 - mb)[:, :, None] * eye).reshape(128, NG * 16).astype(np.float32)

    hostk = np.zeros((128, HK), np.float32)
    hostk[:, HK_TRANS:HK_TRANS + 16] = trans.reshape(-1)[None, :]
    hostk[:, HK_START:HK_START + 4] = start[None, :]
    hostk[:, HK_END:HK_END + 4] = end[None, :]
    hostk[:BL, HK_C0M] = 1.0 - mask[:, 0]
    hostk[:, HK_MB:HK_MB + NG] = mb
    hostk[:, HK_OH4M:HK_OH4M + 128] = oh4m
    hostk[:, HK_GPRE:HK_GPRE + NG] = gpre
    hostk[:, HK_INVI:HK_INVI + 512] = invI
    bias = (np.asarray(shared_b, np.float64)[None, :]
            + np.asarray(domain_b, np.float64)[corpus])         # [BL, T]
    hostk[0, HK_BIAS:HK_BIAS + 128] = np.repeat(
        bias[:, None, :], 4, axis=1).reshape(-1)
    hostk[0, HK_ONESROW:HK_ONESROW + 128] = 1.0
    hostk[:, HK_ONESCOL] = 1.0

    # weights: w8[b, d, t] = domain_A[corpus_b] + shared_W, fp8
    w8 = (np.asarray(domain_A, np.float64)[corpus]
          + np.asarray(shared_W, np.float64)[None]).astype(np.float32)
    w8q = w8.astype(ml_dtypes.float8_e4m3)                      # [BL, D, T]
    rhs8 = np.zeros((128, 6 * NG), ml_dtypes.float8_e4m3)
    dd = np.arange(D)
    if PLAIN8:
        cc, pp_ = dd // 128, dd % 128        # d = c*128 + p
        for b in range(BL):
            rhs8[pp_[:, None], (cc * NG + b * 4)[:, None] + np.arange(T)] = w8q[b]
    else:
        u = dd // 2
        e = dd % 2
        cc, pp_ = u // 128, u % 128          # d = 2*(c2*128+p)+e
        ce = 2 * cc + e
        for b in range(BL):
            rhs8[pp_[:, None], (ce * NG + b * 4)[:, None] + np.arange(T)] = w8q[b]

    return gidx, hostk, rhs8


def kernel(_trace=False, **inputs):
    from concourse.bass_utils import run_bass_kernel_spmd

    words = np.asarray(inputs["words"])
    target = np.asarray(inputs["target"])
    corpus = np.asarray(inputs["corpus"])
    table8 = np.ascontiguousarray(
        np.asarray(inputs["embed_table"], np.float32).astype(ml_dtypes.float8_e4m3))

    nc = _build()
    in_maps = []
    for k in range(NCORES):
        sl = slice(k * BL, (k + 1) * BL)
        gidx, hostk, rhs8 = _prep_core(
            words[sl], target[sl], corpus[sl], inputs["embed_table"],
            inputs["shared_W"], inputs["shared_b"], inputs["domain_A"],
            inputs["domain_b"], inputs["trans_m"], inputs["start_scores"],
            inputs["end_scores"],
        )
        in_maps.append({
            "table": table8, "gidx": gidx, "hostk": hostk, "rhs8": rhs8,
        })
    res = run_bass_kernel_spmd(
        nc, in_maps, core_ids=list(range(NCORES)), trace=_trace,
    )
    if _trace:
        print("exec_time_ns:", res.exec_time_ns,
              "mean:", res.mean_exec_time_ns,
              "trace:", (res.instructions_and_trace or (None, None))[1])
    out = np.concatenate([res.results[k]["nll"] for k in range(NCORES)])
    return out.astype(np.float32)
